# revision 20
# baseline (speedup 1.0000x reference)
"""MoE top-1 routing kernel for Trainium2 (8 NeuronCores, expert-parallel).

Problem: x[65536,1024] fp32; gate = softmax(x @ Wg.T + bg); idx = argmax(gate);
out[n] = x[n] @ We[idx[n]].T + be[idx[n]].

The end-to-end wall time is dominated by the ~35-70 MB/s axon tunnel, so the
design minimizes host<->device bytes:

  Host (cheap: gating GEMM is 2 GFLOP):
    - fp32 routing: logits = x @ Wg.T + bg, idx = argmax (bit-exact fp32, so
      routing matches the reference; device bf16 gating would misroute).
    - per-token int8 quantization of x (per-row absmax scales), into
      preallocated buffers (fresh 256MB allocations cost ~1s/call).
    - counting-sort dispatch: core c owns experts 2c, 2c+1; tokens for each
      expert are packed into a static-capacity slot block (CAP_E per expert).
      Capacity overflow (never hit at these shapes) falls back to host numpy.
  Device (per core, all static, no collectives):
    - 72 token tiles of 128; tiles [0,36) use expert slot 0, [36,72) slot 1.
    - per tile: int8 load -> bf16 convert -> 8 PE transposes (k-major lhsT)
      -> 16 bf16 matmuls (2 psum halves, 8 k-chunks) -> scale by per-token
      input scale (ACT engine) -> +bias -> per-token abs-max -> uint8
      requantize (offset 128) -> store; row scales are a 2nd output.
  Transfers: x int8 (75 MB) + scales + We bf16 pair-sharded (4 MB/core) up;
  out uint8 (75 MB) + row scales down. Donated output buffers are created
  on-device (jnp.zeros jit), not shipped. Weight device buffers are cached
  across calls keyed on array equality.
"""
import os
import time
import threading
import numpy as np
import ml_dtypes

import jax
import jax.numpy as jnp
from jax.sharding import Mesh, PartitionSpec, NamedSharding

import concourse.bass as bass
import concourse.mybir as mybir
import concourse.tile as tile
from concourse import bacc
from concourse import bass2jax as _b2j
from concourse.masks import make_identity

P = 128
N_CORES = 8
N_TOK = 65536
D = 1024                      # d_in = d_out
E = 16
KC = D // P                   # 8 k-chunks
EPC = E // N_CORES            # 2 experts per core
CAP_E = 4224                  # token capacity per expert (33 tiles); overflow
                              # tokens (a few dozen at these shapes) are
                              # computed on host
CAP_C = EPC * CAP_E           # 9216 tokens per core
NTILE = CAP_C // P            # 72
NT_E = CAP_E // P             # 36
QBIAS = 128.0                 # uint8 quant offset (convert rounds to nearest)
QMAX = 126.5                  # max quantized magnitude

FP32 = mybir.dt.float32
FP16 = mybir.dt.float16
BF16 = mybir.dt.bfloat16
I8 = mybir.dt.int8
U8 = mybir.dt.uint8

_STATE: dict = {}


def build_nc():
    nc = bacc.Bacc("TRN2", target_bir_lowering=False, debug=False,
                   enable_asserts=False, num_devices=1)

    xq = nc.dram_tensor("xq", [CAP_C, D], I8, kind="ExternalInput")
    sxT = nc.dram_tensor("sxT", [P, NTILE], FP32, kind="ExternalInput")
    # wePT[s][p][c*D+d] = We[expert(s)][d, c*128+p]  (lhsT layout, host-prepped)
    wePT = nc.dram_tensor("wePT", [EPC, P, KC * D], BF16, kind="ExternalInput")
    beP = nc.dram_tensor("beP", [EPC, P, D], FP32, kind="ExternalInput")
    out = nc.dram_tensor("out", [CAP_C, D], U8, kind="ExternalOutput")
    soT = nc.dram_tensor("soT", [P, NTILE], FP32, kind="ExternalOutput")

    with tile.TileContext(nc) as tc:
        with tc.tile_pool(name="cst", bufs=1) as cst, \
             tc.tile_pool(name="xin", bufs=3) as xin, \
             tc.tile_pool(name="xbp", bufs=2) as xbp, \
             tc.tile_pool(name="gxp", bufs=2) as gxp, \
             tc.tile_pool(name="ofp", bufs=2) as ofp, \
             tc.tile_pool(name="yab", bufs=2) as yap, \
             tc.tile_pool(name="sc", bufs=3) as scp, \
             tc.tile_pool(name="op", bufs=3) as op, \
             tc.tile_pool(name="pt", bufs=4, space="PSUM") as pt, \
             tc.tile_pool(name="pm", bufs=2, space="PSUM") as pm:
            ident = cst.tile([P, P], BF16)
            make_identity(nc, ident[:])
            sx_sb = cst.tile([P, NTILE], FP32)
            nc.sync.dma_start(sx_sb[:], sxT[:])
            so_all = cst.tile([P, NTILE], FP32)
            w_sb = cst.tile([P, EPC, KC, D], BF16)
            for s in range(EPC):
                nc.sync.dma_start(
                    w_sb[:, s, :, :].rearrange("p c d -> p (c d)"), wePT[s])
            be_sb = cst.tile([P, EPC, D], FP32)
            for s in range(EPC):
                nc.sync.dma_start(be_sb[:, s, :], beP[s])

            for t in range(NTILE):
                s = 0 if t < NT_E else 1
                xq_t = xin.tile([P, D], I8, tag="xq")
                nc.sync.dma_start(xq_t[:], xq[t * P:(t + 1) * P, :])
                xbf = xbp.tile([P, D], BF16, tag="xbf")
                nc.vector.tensor_copy(xbf[:], xq_t[:])
                gx = gxp.tile([P, KC, P], BF16, tag="gx")
                for c in range(KC):
                    tp = pt.tile([P, P], BF16, tag="tp")
                    nc.tensor.transpose(tp[:], xbf[:, c * P:(c + 1) * P],
                                        ident[:])
                    nc.vector.tensor_copy(gx[:, c, :], tp[:])
                ps0 = pm.tile([P, 512], FP32, tag="ps0")
                ps1 = pm.tile([P, 512], FP32, tag="ps1")
                for c in range(KC):
                    nc.tensor.matmul(ps0[:], gx[:, c, :],
                                     w_sb[:, s, c, 0:512],
                                     start=(c == 0), stop=(c == KC - 1))
                    nc.tensor.matmul(ps1[:], gx[:, c, :],
                                     w_sb[:, s, c, 512:D],
                                     start=(c == 0), stop=(c == KC - 1))
                # y = psum * s_tok (ACT engine) + be (DVE, in-place fp32)
                of32 = ofp.tile([P, D], FP32, tag="of32")
                nc.scalar.activation(of32[:, 0:512], ps0[:],
                                     mybir.ActivationFunctionType.Copy,
                                     scale=sx_sb[:, t:t + 1])
                nc.scalar.activation(of32[:, 512:D], ps1[:],
                                     mybir.ActivationFunctionType.Copy,
                                     scale=sx_sb[:, t:t + 1])
                nc.vector.tensor_add(of32[:, 0:512], of32[:, 0:512],
                                     be_sb[:, s, 0:512])
                nc.vector.tensor_add(of32[:, 512:D], of32[:, 512:D],
                                     be_sb[:, s, 512:D])
                # per-token abs-max -> scale; requantize to uint8 (+128 offset)
                ya = yap.tile([P, D], FP32, tag="ya")
                nc.scalar.activation(ya[:], of32[:],
                                     mybir.ActivationFunctionType.Abs)
                mx8 = scp.tile([P, 8], FP32, tag="mx8")
                nc.vector.max(mx8[:], ya[:])
                nc.vector.tensor_scalar(so_all[:, t:t + 1], mx8[:, 0:1],
                                        1.0 / QMAX, None,
                                        op0=mybir.AluOpType.mult)
                inv = scp.tile([P, 1], FP32, tag="inv")
                nc.vector.reciprocal(inv[:], so_all[:, t:t + 1])
                o = op.tile([P, D], U8, tag="o")
                nc.scalar.activation(o[:, 0:512], of32[:, 0:512],
                                     mybir.ActivationFunctionType.Copy,
                                     scale=inv[:], bias=QBIAS)
                nc.scalar.activation(o[:, 512:D], of32[:, 512:D],
                                     mybir.ActivationFunctionType.Copy,
                                     scale=inv[:], bias=QBIAS)
                nc.sync.dma_start(out[t * P:(t + 1) * P, :], o[:])
            nc.sync.dma_start(soT[:], so_all[:])

    nc.compile()
    return nc


def _get_state():
    if _STATE.get("ready"):
        return _STATE
    _b2j.install_neuronx_cc_hook()
    nc = build_nc()
    devs = jax.devices()[:N_CORES]
    mesh = Mesh(np.asarray(devs), ("c",))
    shard = NamedSharding(mesh, PartitionSpec("c"))

    partition_name = (nc.partition_id_tensor.name
                      if nc.partition_id_tensor is not None else None)
    in_names, out_names, out_avals = [], [], []
    for alloc in nc.m.functions[0].allocations:
        if not isinstance(alloc, mybir.MemoryLocationSet):
            continue
        name = alloc.memorylocations[0].name
        if alloc.kind == "ExternalInput":
            if name != partition_name:
                in_names.append(name)
        elif alloc.kind == "ExternalOutput":
            out_names.append(name)
            out_avals.append(jax.core.ShapedArray(
                tuple(alloc.tensor_shape), mybir.dt.np(alloc.dtype)))
    n_params = len(in_names)
    all_names = in_names + out_names
    if partition_name is not None:
        all_names = all_names + [partition_name]
    donate = tuple(range(n_params, n_params + len(out_names)))

    def _body(*args):
        operands = list(args)
        if partition_name is not None:
            operands.append(_b2j.partition_id_tensor())
        outs = _b2j._bass_exec_p.bind(
            *operands,
            out_avals=tuple(out_avals),
            in_names=tuple(all_names),
            out_names=tuple(out_names),
            lowering_input_output_aliases=(),
            sim_require_finite=True,
            sim_require_nnan=True,
            nc=nc,
        )
        return tuple(outs)

    from jax.experimental.shard_map import shard_map
    sharded = jax.jit(
        shard_map(_body, mesh=mesh,
                  in_specs=(PartitionSpec("c"),) * (n_params + len(out_names)),
                  out_specs=(PartitionSpec("c"),) * len(out_names),
                  check_rep=False),
        donate_argnums=donate, keep_unused=True)

    # per-device variant for pipelined upload/exec/download
    single = jax.jit(_body, donate_argnums=donate, keep_unused=True)

    zeros_jit = jax.jit(
        lambda: tuple(jnp.zeros((N_CORES * a.shape[0],) + a.shape[1:], a.dtype)
                      for a in out_avals),
        out_shardings=tuple(shard for _ in out_avals))

    _STATE.update(ready=True, nc=nc, devs=devs, mesh=mesh, shard=shard,
                  in_names=in_names, out_names=out_names, out_avals=out_avals,
                  sharded=sharded, single=single, zeros_jit=zeros_jit,
                  wcache=None,
                  xq_bufs=[np.empty((CAP_C, D), np.int8)
                           for _ in range(N_CORES)],
                  qbuf=np.empty((CAP_E, D), np.float32),
                  y=np.empty((N_TOK, D), np.float32))
    return _STATE


def _global_from_shards(st, shards, shape, dtype):
    """Assemble a sharded global jax array from 8 per-device host arrays."""
    arrs = [jax.device_put(shards[c], st["devs"][c]) for c in range(N_CORES)]
    gshape = (N_CORES * shape[0],) + tuple(shape[1:])
    return jax.make_array_from_single_device_arrays(gshape, st["shard"], arrs)


def _prep_weights(st, Wg, bg, We, be):
    """Device-resident wePT/beP, cached across calls on array equality."""
    wc = st.get("wcache")
    if wc is not None and np.array_equal(wc["We"], We) and \
            np.array_equal(wc["be"], be):
        return wc["wePT_g"], wc["beP_g"]
    # wePT[e][p][c*D+d] = We[e][d, c*128+p]
    weT = We.transpose(0, 2, 1)                            # [E, k, d]
    wePT = np.ascontiguousarray(
        weT.reshape(E, KC, P, D).transpose(0, 2, 1, 3).reshape(E, P, KC * D)
    ).astype(ml_dtypes.bfloat16)
    beP = np.ascontiguousarray(
        np.broadcast_to(be[:, None, :], (E, P, D))).astype(np.float32)
    wePT_g = _global_from_shards(
        st, [wePT[c * EPC:(c + 1) * EPC] for c in range(N_CORES)],
        (EPC, P, KC * D), ml_dtypes.bfloat16)
    beP_g = _global_from_shards(
        st, [beP[c * EPC:(c + 1) * EPC] for c in range(N_CORES)],
        (EPC, P, D), np.float32)
    st["wcache"] = dict(We=We.copy(), be=be.copy(), wePT_g=wePT_g, beP_g=beP_g)
    return wePT_g, beP_g


def kernel(x, Wg, bg, We, be):
    tt = [("start", time.time())]

    def _tick(name):
        tt.append((name, time.time()))

    x = np.asarray(x, dtype=np.float32)
    Wg = np.asarray(Wg, dtype=np.float32)
    bg = np.asarray(bg, dtype=np.float32)
    We = np.asarray(We, dtype=np.float32)
    be = np.asarray(be, dtype=np.float32)
    assert x.shape == (N_TOK, D) and We.shape == (E, D, D), (x.shape, We.shape)

    st = _get_state()
    _tick("state")
    wePT_g, beP_g = _prep_weights(st, Wg, bg, We, be)
    _tick("weights")

    # ---- fp32 routing on host (matches reference bit-for-bit in practice)
    logits = x @ Wg.T
    logits += bg
    idx = np.argmax(logits, axis=1).astype(np.int32)
    _tick("routing")

    # ---- per-token int8 scales (the quantization itself happens per-core,
    # overlapped with uploads)
    s = np.abs(x).max(axis=1)
    s /= 127.0
    np.maximum(s, 1e-30, out=s)
    inv_s = 1.0 / s
    _tick("quant")

    # ---- dispatch: slot tables per core (expert e -> core e//2, slot e%2)
    order = np.argsort(idx, kind="stable")
    counts = np.bincount(idx, minlength=E)
    starts = np.zeros(E + 1, np.int64)
    np.cumsum(counts, out=starts[1:])
    tok_by_e = [order[starts[e]:starts[e + 1]] for e in range(E)]
    overflow = []                                  # (expert, token-array)
    for e in range(E):
        if counts[e] > CAP_E:
            overflow.append((e, tok_by_e[e][CAP_E:]))
            tok_by_e[e] = tok_by_e[e][:CAP_E]

    # ---- pipelined per-core: build -> upload -> exec -> download, overlapped
    devs = st["devs"]
    zeros = st["zeros_jit"]()
    zparts = [sorted(z.addressable_shards, key=lambda sd: sd.index[0].start)
              for z in zeros]
    wparts = sorted(wePT_g.addressable_shards, key=lambda sd: sd.index[0].start)
    bparts = sorted(beP_g.addressable_shards, key=lambda sd: sd.index[0].start)
    name_pos = {n: i for i, n in enumerate(st["in_names"])}
    single = st["single"]
    out_pos = {n: i for i, n in enumerate(st["out_names"])}

    y = st["y"]
    fetch_t = np.zeros(N_CORES)
    scat_t = np.zeros(N_CORES)
    outs_pc = [None] * N_CORES
    threads = []

    def _fetch(c):
        t0 = time.time()
        part = np.asarray(outs_pc[c][out_pos["out"]])   # [CAP_C, D] uint8
        soT = np.asarray(outs_pc[c][out_pos["soT"]])    # [P, NTILE] fp32
        t1 = time.time()
        so = soT.T.reshape(CAP_C)
        for sl in range(EPC):
            tk = tok_by_e[c * EPC + sl]
            n = len(tk)
            blk = part[sl * CAP_E:sl * CAP_E + n].astype(np.int16)
            blk -= 128
            y[tk] = blk * so[sl * CAP_E:sl * CAP_E + n, None]
        fetch_t[c] = t1 - t0
        scat_t[c] = time.time() - t1

    xq_bufs = st["xq_bufs"]
    qbuf = st["qbuf"]
    for c in range(N_CORES):
        xq_pad = xq_bufs[c]
        s_pad = np.zeros(CAP_C, np.float32)
        for sl in range(EPC):
            tk = tok_by_e[c * EPC + sl]
            n = len(tk)
            # gather fp32 rows, quantize into the padded int8 buffer
            np.multiply(x[tk], inv_s[tk, None], out=qbuf[:n])
            np.rint(qbuf[:n], out=qbuf[:n])
            np.copyto(xq_pad[sl * CAP_E:sl * CAP_E + n], qbuf[:n],
                      casting="unsafe")
            xq_pad[sl * CAP_E + n:(sl + 1) * CAP_E] = 0
            s_pad[sl * CAP_E:sl * CAP_E + n] = s[tk]
        sxT = np.ascontiguousarray(s_pad.reshape(NTILE, P).T)
        args = [None] * len(st["in_names"])
        args[name_pos["xq"]] = jax.device_put(xq_pad, devs[c])
        args[name_pos["sxT"]] = jax.device_put(sxT, devs[c])
        args[name_pos["wePT"]] = wparts[c].data
        args[name_pos["beP"]] = bparts[c].data
        outs_pc[c] = single(*args, *[zp[c].data for zp in zparts])
        th = threading.Thread(target=_fetch, args=(c,))
        th.start()
        threads.append(th)
        if c == 0:
            _tick("dispatch_build")
    for t in threads:
        t.join()
    _tick("exec_download")

    # ---- host fallback for capacity overflow (a few dozen rows)
    for e, tk in overflow:
        y[tk] = x[tk] @ We[e].T + be[e]

    _tick("download_scatter")
    kernel.last_results = None
    if os.environ.get("MOE_TIME"):
        for (n0, t0), (n1, t1) in zip(tt, tt[1:]):
            print(f"  [{n1}] {t1 - t0:.3f}s")
        print(f"  [total] {tt[-1][1] - tt[0][1]:.3f}s")
        print(f"  fetch={fetch_t.sum():.3f}s(sum) scat={scat_t.sum():.3f}s(sum)")
    return y


# revision 21
# speedup vs baseline: 1.0669x; 1.0669x over previous
"""MoE top-1 routing kernel for Trainium2 (8 NeuronCores, expert-parallel).

Problem: x[65536,1024] fp32; gate = softmax(x @ Wg.T + bg); idx = argmax(gate);
out[n] = x[n] @ We[idx[n]].T + be[idx[n]].

End-to-end wall time is dominated by the axon tunnel (~40 MB/s per process,
scales ~linearly with processes), so the design minimizes bytes AND
parallelizes the tunnel across 8 worker processes (one NeuronCore each,
shared-memory IPC):

  Main process (no device work):
    - fp32 routing: logits = x @ Wg.T + bg, idx = argmax (bit-exact fp32 so
      routing matches the reference; bf16/fp16 gating would misroute).
    - counting-sort dispatch: core c owns experts 2c, 2c+1, each with a
      static CAP_E-token slot block; per-token int8 quantization (per-row
      absmax scales) written straight into shared memory; capacity overflow
      (a few dozen rows at these shapes) is computed on host.
  Worker process c (own jax/axon connection -> own tunnel bandwidth):
    - device_put int8 tokens + scales, run the bass kernel on core c,
      download uint8 outputs + per-token scales, dequant-scatter into the
      shared fp32 output. Weights (bf16, pair-sharded) are uploaded once
      and cached on device across calls.
  Device kernel (per core, all static, no collectives):
    - 66 token tiles of 128; tiles [0,33) use expert slot 0, rest slot 1.
    - per tile: int8 load -> bf16 convert -> 8 PE transposes (k-major lhsT)
      -> 16 bf16 matmuls (2 psum halves, 8 k-chunks) -> scale by per-token
      input scale (ACT engine) -> +bias -> per-token abs-max -> uint8
      requantize (offset 128, round-to-nearest) -> store; row scales are a
      second output. Donated output buffers are created on-device.

A single-process fallback path (MOE_WORKERS=0 or worker failure) runs the
same flow inline over all 8 cores.
"""
import atexit
import os
import time
import threading
import multiprocessing as mp
from multiprocessing import shared_memory
import numpy as np
import ml_dtypes

import jax
import jax.numpy as jnp

P = 128
N_CORES = 8
N_TOK = 65536
D = 1024                      # d_in = d_out
E = 16
KC = D // P                   # 8 k-chunks
EPC = E // N_CORES            # 2 experts per core
CAP_E = 4224                  # token capacity per expert (33 tiles); overflow
                              # tokens (a few dozen at these shapes) are
                              # computed on host
CAP_C = EPC * CAP_E           # tokens per core
NTILE = CAP_C // P            # 66
NT_E = CAP_E // P             # 33
QBIAS = 128.0                 # uint8 quant offset (convert rounds to nearest)
QMAX = 126.5                  # max quantized magnitude

_STATE: dict = {}             # per-process lazy state


# --------------------------------------------------------------------------
# device kernel
# --------------------------------------------------------------------------

def build_nc():
    import concourse.mybir as mybir
    import concourse.tile as tile
    from concourse import bacc
    from concourse.masks import make_identity

    FP32 = mybir.dt.float32
    BF16 = mybir.dt.bfloat16
    I8 = mybir.dt.int8
    U8 = mybir.dt.uint8

    nc = bacc.Bacc("TRN2", target_bir_lowering=False, debug=False,
                   enable_asserts=False, num_devices=1)

    xq = nc.dram_tensor("xq", [CAP_C, D], I8, kind="ExternalInput")
    sxT = nc.dram_tensor("sxT", [P, NTILE], FP32, kind="ExternalInput")
    # wePT[s][p][c*D+d] = We[expert(s)][d, c*128+p]  (lhsT layout, host-prepped)
    wePT = nc.dram_tensor("wePT", [EPC, P, KC * D], BF16, kind="ExternalInput")
    beP = nc.dram_tensor("beP", [EPC, P, D], FP32, kind="ExternalInput")
    out = nc.dram_tensor("out", [CAP_C, D], U8, kind="ExternalOutput")
    soT = nc.dram_tensor("soT", [P, NTILE], FP32, kind="ExternalOutput")

    with tile.TileContext(nc) as tc:
        with tc.tile_pool(name="cst", bufs=1) as cst, \
             tc.tile_pool(name="xin", bufs=3) as xin, \
             tc.tile_pool(name="xbp", bufs=2) as xbp, \
             tc.tile_pool(name="gxp", bufs=2) as gxp, \
             tc.tile_pool(name="ofp", bufs=2) as ofp, \
             tc.tile_pool(name="yab", bufs=2) as yap, \
             tc.tile_pool(name="sc", bufs=3) as scp, \
             tc.tile_pool(name="op", bufs=3) as op, \
             tc.tile_pool(name="pt", bufs=4, space="PSUM") as pt, \
             tc.tile_pool(name="pm", bufs=2, space="PSUM") as pm:
            ident = cst.tile([P, P], BF16)
            make_identity(nc, ident[:])
            sx_sb = cst.tile([P, NTILE], FP32)
            nc.sync.dma_start(sx_sb[:], sxT[:])
            so_all = cst.tile([P, NTILE], FP32)
            w_sb = cst.tile([P, EPC, KC, D], BF16)
            for s in range(EPC):
                nc.sync.dma_start(
                    w_sb[:, s, :, :].rearrange("p c d -> p (c d)"), wePT[s])
            be_sb = cst.tile([P, EPC, D], FP32)
            for s in range(EPC):
                nc.sync.dma_start(be_sb[:, s, :], beP[s])

            for t in range(NTILE):
                s = 0 if t < NT_E else 1
                xq_t = xin.tile([P, D], I8, tag="xq")
                nc.sync.dma_start(xq_t[:], xq[t * P:(t + 1) * P, :])
                xbf = xbp.tile([P, D], BF16, tag="xbf")
                nc.vector.tensor_copy(xbf[:], xq_t[:])
                gx = gxp.tile([P, KC, P], BF16, tag="gx")
                for c in range(KC):
                    tp = pt.tile([P, P], BF16, tag="tp")
                    nc.tensor.transpose(tp[:], xbf[:, c * P:(c + 1) * P],
                                        ident[:])
                    nc.vector.tensor_copy(gx[:, c, :], tp[:])
                ps0 = pm.tile([P, 512], FP32, tag="ps0")
                ps1 = pm.tile([P, 512], FP32, tag="ps1")
                for c in range(KC):
                    nc.tensor.matmul(ps0[:], gx[:, c, :],
                                     w_sb[:, s, c, 0:512],
                                     start=(c == 0), stop=(c == KC - 1))
                    nc.tensor.matmul(ps1[:], gx[:, c, :],
                                     w_sb[:, s, c, 512:D],
                                     start=(c == 0), stop=(c == KC - 1))
                # y = psum * s_tok (ACT engine) + be (DVE, in-place fp32)
                of32 = ofp.tile([P, D], FP32, tag="of32")
                nc.scalar.activation(of32[:, 0:512], ps0[:],
                                     mybir.ActivationFunctionType.Copy,
                                     scale=sx_sb[:, t:t + 1])
                nc.scalar.activation(of32[:, 512:D], ps1[:],
                                     mybir.ActivationFunctionType.Copy,
                                     scale=sx_sb[:, t:t + 1])
                nc.vector.tensor_add(of32[:, 0:512], of32[:, 0:512],
                                     be_sb[:, s, 0:512])
                nc.vector.tensor_add(of32[:, 512:D], of32[:, 512:D],
                                     be_sb[:, s, 512:D])
                # per-token abs-max -> scale; requantize to uint8 (+128)
                ya = yap.tile([P, D], FP32, tag="ya")
                nc.scalar.activation(ya[:], of32[:],
                                     mybir.ActivationFunctionType.Abs)
                mx8 = scp.tile([P, 8], FP32, tag="mx8")
                nc.vector.max(mx8[:], ya[:])
                nc.vector.tensor_scalar(so_all[:, t:t + 1], mx8[:, 0:1],
                                        1.0 / QMAX, None,
                                        op0=mybir.AluOpType.mult)
                inv = scp.tile([P, 1], FP32, tag="inv")
                nc.vector.reciprocal(inv[:], so_all[:, t:t + 1])
                o = op.tile([P, D], U8, tag="o")
                nc.scalar.activation(o[:, 0:512], of32[:, 0:512],
                                     mybir.ActivationFunctionType.Copy,
                                     scale=inv[:], bias=QBIAS)
                nc.scalar.activation(o[:, 512:D], of32[:, 512:D],
                                     mybir.ActivationFunctionType.Copy,
                                     scale=inv[:], bias=QBIAS)
                nc.sync.dma_start(out[t * P:(t + 1) * P, :], o[:])
            nc.sync.dma_start(soT[:], so_all[:])

    nc.compile()
    return nc


# --------------------------------------------------------------------------
# per-process execution state (used by workers and by the inline fallback)
# --------------------------------------------------------------------------

def _build_exec_state():
    """nc + jit wrappers; shared by worker processes and inline fallback."""
    import concourse.mybir as mybir
    from concourse import bass2jax as _b2j

    _b2j.install_neuronx_cc_hook()
    nc = build_nc()

    partition_name = (nc.partition_id_tensor.name
                      if nc.partition_id_tensor is not None else None)
    in_names, out_names, out_avals = [], [], []
    for alloc in nc.m.functions[0].allocations:
        if not isinstance(alloc, mybir.MemoryLocationSet):
            continue
        name = alloc.memorylocations[0].name
        if alloc.kind == "ExternalInput":
            if name != partition_name:
                in_names.append(name)
        elif alloc.kind == "ExternalOutput":
            out_names.append(name)
            out_avals.append(jax.core.ShapedArray(
                tuple(alloc.tensor_shape), mybir.dt.np(alloc.dtype)))
    n_params = len(in_names)
    all_names = in_names + out_names
    if partition_name is not None:
        all_names = all_names + [partition_name]
    donate = tuple(range(n_params, n_params + len(out_names)))

    def _body(*args):
        operands = list(args)
        if partition_name is not None:
            operands.append(_b2j.partition_id_tensor())
        outs = _b2j._bass_exec_p.bind(
            *operands,
            out_avals=tuple(out_avals),
            in_names=tuple(all_names),
            out_names=tuple(out_names),
            lowering_input_output_aliases=(),
            sim_require_finite=True,
            sim_require_nnan=True,
            nc=nc,
        )
        return tuple(outs)

    single = jax.jit(_body, donate_argnums=donate, keep_unused=True)
    return dict(nc=nc, in_names=in_names, out_names=out_names,
                out_avals=out_avals, single=single)


def _core_zeros(es, dev):
    from jax.sharding import SingleDeviceSharding
    sh = SingleDeviceSharding(dev)
    fn = jax.jit(
        lambda: tuple(jnp.zeros(a.shape, a.dtype) for a in es["out_avals"]),
        out_shardings=tuple(sh for _ in es["out_avals"]))
    return fn


def _prep_weights_host(We, be):
    """wePT[e][p][c*D+d] = We[e][d, c*128+p]; beP broadcast over partitions."""
    weT = We.transpose(0, 2, 1)                            # [E, k, d]
    wePT = np.ascontiguousarray(
        weT.reshape(E, KC, P, D).transpose(0, 2, 1, 3).reshape(E, P, KC * D)
    ).astype(ml_dtypes.bfloat16)
    beP = np.ascontiguousarray(
        np.broadcast_to(be[:, None, :], (E, P, D))).astype(np.float32)
    return wePT, beP


def _run_core(es, dev, zeros_fn, xq_view, sx_view, w_args, y, tok_lists,
              x_overflow=None):
    """Upload one core's tokens, execute, download, dequant-scatter into y."""
    name_pos = {n: i for i, n in enumerate(es["in_names"])}
    out_pos = {n: i for i, n in enumerate(es["out_names"])}
    args = [None] * len(es["in_names"])
    args[name_pos["xq"]] = jax.device_put(xq_view, dev)
    args[name_pos["sxT"]] = jax.device_put(sx_view, dev)
    args[name_pos["wePT"]] = w_args[0]
    args[name_pos["beP"]] = w_args[1]
    outs = es["single"](*args, *zeros_fn())
    part = np.asarray(outs[out_pos["out"]])      # [CAP_C, D] uint8
    soT = np.asarray(outs[out_pos["soT"]])       # [P, NTILE] fp32
    so = soT.T.reshape(CAP_C)
    for sl in range(EPC):
        tk = tok_lists[sl]
        n = len(tk)
        if n == 0:
            continue
        blk = part[sl * CAP_E:sl * CAP_E + n].astype(np.int16)
        blk -= 128
        y[tk] = blk * so[sl * CAP_E:sl * CAP_E + n, None]


def _quant_block(x, tk, xq_dst, sx_dst, qbuf):
    """Gather rows tk of x, quantize to int8 into xq_dst, scales to sx_dst."""
    n = len(tk)
    xs = qbuf[:n]
    np.take(x, tk, axis=0, out=xs)
    s = np.abs(xs).max(axis=1)
    s /= 127.0
    np.maximum(s, 1e-30, out=s)
    np.multiply(xs, (1.0 / s)[:, None], out=xs)
    np.rint(xs, out=xs)
    np.copyto(xq_dst[:n], xs, casting="unsafe")
    xq_dst[n:] = 0
    sx_dst[:n] = s
    sx_dst[n:] = 0.0
    return s


def _route(x, Wg, bg):
    logits = x @ Wg.T
    logits += bg
    idx = np.argmax(logits, axis=1).astype(np.int32)
    order = np.argsort(idx, kind="stable").astype(np.int32)
    counts = np.bincount(idx, minlength=E).astype(np.int64)
    starts = np.zeros(E + 1, np.int64)
    np.cumsum(counts, out=starts[1:])
    return order, counts, starts


# --------------------------------------------------------------------------
# worker process
# --------------------------------------------------------------------------

def _worker_main(core, shm_names, conn):
    try:
        shms = {k: shared_memory.SharedMemory(name=v)
                for k, v in shm_names.items()}
        XQ = np.ndarray((N_CORES, CAP_C, D), np.int8, buffer=shms["XQ"].buf)
        SX = np.ndarray((N_CORES, P, NTILE), np.float32, buffer=shms["SX"].buf)
        W = np.ndarray((E, P, KC * D), np.uint16, buffer=shms["W"].buf)
        BE = np.ndarray((E, P, D), np.float32, buffer=shms["BE"].buf)
        ORD = np.ndarray((N_TOK,), np.int32, buffer=shms["ORD"].buf)
        CNT = np.ndarray((E,), np.int64, buffer=shms["CNT"].buf)
        STF = np.ndarray((E + 1,), np.int64, buffer=shms["STF"].buf)
        Y = np.ndarray((N_TOK, D), np.float32, buffer=shms["Y"].buf)
        conn.send(("booted", core))

        es = None
        zeros_fn = None
        dev = None
        w_args = None
        wver_seen = -1
        while True:
            msg = conn.recv()
            if msg[0] == "quit":
                break
            if msg[0] == "init":
                es = _build_exec_state()
                dev = jax.devices()[core]
                zeros_fn = _core_zeros(es, dev)
                # warmup: trigger jit wrapper compiles with dummy data
                dummy_y = np.empty((N_TOK, D), np.float32)
                tk0 = np.arange(4, dtype=np.int32)
                _run_core(es, dev, zeros_fn, XQ[core], SX[core],
                          (jax.device_put(
                              W[core * EPC:(c0 := core * EPC) + EPC].view(
                                  ml_dtypes.bfloat16), dev),
                           jax.device_put(BE[c0:c0 + EPC], dev)),
                          dummy_y, [tk0, tk0])
                conn.send(("ready", core))
            elif msg[0] == "run":
                wver = msg[1]
                if wver != wver_seen:
                    c0 = core * EPC
                    w_args = (
                        jax.device_put(
                            W[c0:c0 + EPC].view(ml_dtypes.bfloat16), dev),
                        jax.device_put(BE[c0:c0 + EPC], dev))
                    wver_seen = wver
                tok_lists = []
                for sl in range(EPC):
                    e = core * EPC + sl
                    tok_lists.append(ORD[STF[e]:STF[e] + CNT[e]])
                _run_core(es, dev, zeros_fn, XQ[core], SX[core], w_args,
                          Y, tok_lists)
                conn.send(("done", core))
    except Exception as ex:  # pragma: no cover
        try:
            import traceback
            conn.send(("error", core, f"{ex!r}\n{traceback.format_exc()}"))
        except Exception:
            pass


# --------------------------------------------------------------------------
# main-process orchestration
# --------------------------------------------------------------------------

def _cleanup_shm(st):
    for proc, conn in zip(st.get("procs", []), st.get("conns", [])):
        try:
            conn.send(("quit",))
        except Exception:
            pass
    for proc in st.get("procs", []):
        try:
            proc.join(timeout=2)
            if proc.is_alive():
                proc.terminate()
        except Exception:
            pass
    for shm in st.get("shms", {}).values():
        try:
            shm.close()
            shm.unlink()
        except Exception:
            pass


def _spawn_workers(st):
    ctx = mp.get_context("spawn")
    spec = dict(
        XQ=N_CORES * CAP_C * D,                       # int8
        SX=N_CORES * P * NTILE * 4,                   # f32
        W=E * P * KC * D * 2,                         # bf16 (as uint16)
        BE=E * P * D * 4,                             # f32
        ORD=N_TOK * 4,                                # i32
        CNT=E * 8,                                    # i64
        STF=(E + 1) * 8,                              # i64
        Y=N_TOK * D * 4,                              # f32
    )
    shms = {k: shared_memory.SharedMemory(create=True, size=v)
            for k, v in spec.items()}
    names = {k: s.name for k, s in shms.items()}
    procs, conns = [], []
    for core in range(N_CORES):
        pc, cc = ctx.Pipe()
        p = ctx.Process(target=_worker_main, args=(core, names, cc),
                        daemon=True)
        p.start()
        procs.append(p)
        conns.append(pc)
    st.update(shms=shms, procs=procs, conns=conns,
              XQ=np.ndarray((N_CORES, CAP_C, D), np.int8, buffer=shms["XQ"].buf),
              SX=np.ndarray((N_CORES, P, NTILE), np.float32, buffer=shms["SX"].buf),
              W=np.ndarray((E, P, KC * D), np.uint16, buffer=shms["W"].buf),
              BE=np.ndarray((E, P, D), np.float32, buffer=shms["BE"].buf),
              ORD=np.ndarray((N_TOK,), np.int32, buffer=shms["ORD"].buf),
              CNT=np.ndarray((E,), np.int64, buffer=shms["CNT"].buf),
              STF=np.ndarray((E + 1,), np.int64, buffer=shms["STF"].buf),
              Y=np.ndarray((N_TOK, D), np.float32, buffer=shms["Y"].buf))
    atexit.register(_cleanup_shm, st)
    for conn in conns:
        msg = conn.recv()
        if msg[0] != "booted":
            raise RuntimeError(f"worker boot failed: {msg}")


def _workers_init(st, timeout=600):
    for conn in st["conns"]:
        conn.send(("init",))
    for conn in st["conns"]:
        if not conn.poll(timeout):
            raise RuntimeError("worker init timeout")
        msg = conn.recv()
        if msg[0] != "ready":
            raise RuntimeError(f"worker init failed: {msg}")


def _get_main_state():
    if _STATE.get("main_ready"):
        return _STATE
    n_workers = int(os.environ.get("MOE_WORKERS", str(N_CORES)))
    _STATE.update(main_ready=True, n_workers=n_workers, wver=0,
                  wcache_key=None, workers_up=False,
                  qbuf=np.empty((CAP_E, D), np.float32))
    return _STATE


def _ensure_workers(st):
    if st["workers_up"]:
        return
    _spawn_workers(st)
    st["workers_up"] = True
    st["workers_inited"] = False


def _kernel_workers(st, x, Wg, bg, We, be, tt):
    if not st["workers_up"]:
        _ensure_workers(st)

    # weights into shm on change
    key = (We.ctypes.data, be.ctypes.data, We.shape)
    if st["wcache_key"] is None or not (
            np.array_equal(st["_We"], We) and np.array_equal(st["_be"], be)):
        wePT, beP = _prep_weights_host(We, be)
        st["W"][:] = wePT.view(np.uint16)
        st["BE"][:] = beP
        st["_We"] = We.copy()
        st["_be"] = be.copy()
        st["wcache_key"] = key
        st["wver"] += 1
    tt.append(("weights", time.time()))

    if not st.get("workers_inited"):
        _workers_init(st)
        st["workers_inited"] = True
        tt.append(("worker_init", time.time()))

    # routing
    order, counts, starts = _route(x, Wg, bg)
    st["ORD"][:] = order
    st["STF"][:] = starts
    capped = np.minimum(counts, CAP_E)
    st["CNT"][:] = capped
    overflow = [(e, order[starts[e] + CAP_E:starts[e + 1]])
                for e in range(E) if counts[e] > CAP_E]
    tt.append(("routing", time.time()))

    # per-core quantize into shm, signal workers as each core is ready
    conns = st["conns"]
    qbuf = st["qbuf"]
    XQ, SX = st["XQ"], st["SX"]
    s_pad = np.empty(CAP_C, np.float32)
    for c in range(N_CORES):
        for sl in range(EPC):
            e = c * EPC + sl
            tk = order[starts[e]:starts[e] + capped[e]]
            _quant_block(x, tk, XQ[c, sl * CAP_E:(sl + 1) * CAP_E],
                         s_pad[sl * CAP_E:(sl + 1) * CAP_E], qbuf)
        SX[c][:] = s_pad.reshape(NTILE, P).T
        conns[c].send(("run", st["wver"]))
    tt.append(("quant_dispatch", time.time()))

    # wait for workers
    errs = []
    for c in range(N_CORES):
        if not conns[c].poll(600):
            raise RuntimeError(f"worker {c} timeout")
        msg = conns[c].recv()
        if msg[0] != "done":
            errs.append(msg)
    if errs:
        raise RuntimeError(f"worker errors: {errs}")
    tt.append(("exec_download", time.time()))

    y = st["Y"]
    for e, tk in overflow:
        y[tk] = x[tk] @ We[e].T + be[e]
    tt.append(("overflow", time.time()))
    return y


def _kernel_inline(st, x, Wg, bg, We, be, tt):
    """Single-process fallback: same flow, all 8 cores from this process."""
    es = st.get("es")
    if es is None:
        es = st["es"] = _build_exec_state()
        devs = jax.devices()[:N_CORES]
        st["es_devs"] = devs
        st["es_zeros"] = [_core_zeros(es, d) for d in devs]
        st["es_wv"] = None
        st["es_y"] = np.empty((N_TOK, D), np.float32)
        st["es_qbuf"] = np.empty((CAP_E, D), np.float32)
        st["es_xq"] = [np.empty((CAP_C, D), np.int8) for _ in range(N_CORES)]
        st["es_sx"] = [np.empty((P, NTILE), np.float32) for _ in range(N_CORES)]
    devs = st["es_devs"]

    if st["es_wv"] is None or not (
            np.array_equal(st["es_We"], We) and np.array_equal(st["es_be"], be)):
        wePT, beP = _prep_weights_host(We, be)
        st["es_wv"] = [
            (jax.device_put(wePT[c * EPC:(c + 1) * EPC], devs[c]),
             jax.device_put(beP[c * EPC:(c + 1) * EPC], devs[c]))
            for c in range(N_CORES)]
        st["es_We"] = We.copy()
        st["es_be"] = be.copy()
    tt.append(("weights", time.time()))

    order, counts, starts = _route(x, Wg, bg)
    capped = np.minimum(counts, CAP_E)
    overflow = [(e, order[starts[e] + CAP_E:starts[e + 1]])
                for e in range(E) if counts[e] > CAP_E]
    tt.append(("routing", time.time()))

    y = st["es_y"]
    threads = []
    s_pad = np.empty(CAP_C, np.float32)
    for c in range(N_CORES):
        tok_lists = []
        for sl in range(EPC):
            e = c * EPC + sl
            tk = order[starts[e]:starts[e] + capped[e]]
            tok_lists.append(tk)
            _quant_block(x, tk, st["es_xq"][c][sl * CAP_E:(sl + 1) * CAP_E],
                         s_pad[sl * CAP_E:(sl + 1) * CAP_E], st["es_qbuf"])
        st["es_sx"][c][:] = s_pad.reshape(NTILE, P).T
        th = threading.Thread(
            target=_run_core,
            args=(es, devs[c], st["es_zeros"][c], st["es_xq"][c],
                  st["es_sx"][c], st["es_wv"][c], y, tok_lists))
        th.start()
        threads.append(th)
    for th in threads:
        th.join()
    tt.append(("exec_download", time.time()))

    for e, tk in overflow:
        y[tk] = x[tk] @ We[e].T + be[e]
    tt.append(("overflow", time.time()))
    return y


def kernel(x, Wg, bg, We, be):
    tt = [("start", time.time())]
    x = np.asarray(x, dtype=np.float32)
    Wg = np.asarray(Wg, dtype=np.float32)
    bg = np.asarray(bg, dtype=np.float32)
    We = np.asarray(We, dtype=np.float32)
    be = np.asarray(be, dtype=np.float32)
    assert x.shape == (N_TOK, D) and We.shape == (E, D, D), (x.shape, We.shape)

    st = _get_main_state()
    if st["n_workers"] > 0:
        try:
            y = _kernel_workers(st, x, Wg, bg, We, be, tt)
        except Exception as ex:
            import traceback
            print(f"[kernel] worker path failed ({ex!r}), falling back inline")
            traceback.print_exc()
            st["n_workers"] = 0
            tt.append(("worker_fail", time.time()))
            y = _kernel_inline(st, x, Wg, bg, We, be, tt)
    else:
        y = _kernel_inline(st, x, Wg, bg, We, be, tt)

    kernel.last_results = None
    if os.environ.get("MOE_TIME"):
        for (n0, t0), (n1, t1) in zip(tt, tt[1:]):
            print(f"  [{n1}] {t1 - t0:.3f}s")
        print(f"  [total] {tt[-1][1] - tt[0][1]:.3f}s")
    return y
